# revision 12
# baseline (speedup 1.0000x reference)
"""Boundary-calculation module (4 fixed 3x3 Sobel-like kernels -> sqrt-sum-sq -> sigmoid)
as a Trainium2 Bass kernel, data-parallel over 8 NeuronCores (batch 32 -> 4 images/core).

Math: with integer kernels K_k (reference kernels x4), the output is
    out = sigmoid(sqrt(SS)/8),  SS = E0^2 + E1^2 + E2^2 + E3^2
and the filter bank is rotated into four *separable* filters (exact identity):
    SS = f0^2 + f1^2 + g2^2 + g3^2
    f0 = diffv(smoothh(x))          (= E0)
    f1 = smoothv(diffh(x))          (= E1)
    g2 = sqrt(2)*diffv(boxh(x))
    g3 = sqrt(2)*boxv(diffh(x))
Vertical 3-tap convs run on the TensorEngine as banded matmuls (lhsT = band
matrix); horizontal shifts are free rhs access-pattern offsets; diffh runs once
on the VectorEngine. Squares/sums on ACT+DVE, sqrt on ACT (sqrt_and_others
table set), sigmoids batched at the end (sigmoid_and_others, single switch).
"""

import os
import sys

sys.path.insert(0, "/opt/trn_rl_repo")

import numpy as np

import concourse.bacc as bacc
import concourse.bass as bass
import concourse.mybir as mybir
from concourse.tile import TileContext, add_dep_helper

AF = mybir.ActivationFunctionType
F32 = mybir.dt.float32
BF16 = mybir.dt.bfloat16

KERNEL_V = int(os.environ.get("KERNEL_V", "6"))


def _register_custom_ops():
    """Register SQUARE_PS_ANT (out = in0^2) and SQUARE_ADD_ANT
    (out = in0^2 + in1) as custom DVE ops at runtime. They let the DVE read
    each PSUM map once and produce the running sum-of-squares without any
    ScalarE Square passes."""
    import concourse.dve_ops as dops
    from concourse.dve_spec import Spec, Src0, Src1, C0, C1, C2, lower, _has_src1
    from concourse.dve_uop import DveOpSpec

    if "SQUARE_PS_ANT" in dops._SUB_OPCODE_FOR_NAME:
        return (
            dops._BY_NAME_ANT["SQUARE_PS_ANT"],
            dops._BY_NAME_ANT["SQUARE_ADD_ANT"],
            dops._BY_NAME_ANT["SIGMOID_POLY_ANT"],
        )

    def make(name, row, spec):
        dops._SUB_OPCODE_FOR_NAME[name] = row
        shas = {}
        for ver in ("v3", "v4"):
            try:
                compiled = DveOpSpec(
                    name=name,
                    opcode=row,
                    uops=lower(spec, ver=ver),
                    rd1_en=_has_src1(spec),
                )
                shas[ver] = compiled.sha(ver)
            except Exception:
                pass
        op = dops.DveOp(name, spec, False, shas)
        dops.OPS.append(op)
        dops.CUSTOM_DVE_SPECS[name] = spec
        return op

    next_row = max(dops._SUB_OPCODE_FOR_NAME.values()) + 1
    # NB: sq(Src0) lowers to something the DVE firmware rejects
    # (NRT_EXEC_UNIT_UNRECOVERABLE on HW); Src0*Src0 works.
    sq_op = make(
        "SQUARE_PS_ANT",
        next_row,
        Spec(
            body=Src0 * Src0,
            reference=lambda in0, in1, s0, s1, imm2: (
                in0.astype(np.float32) ** 2
            ).astype(np.float32),
        ),
    )
    sqa_op = make(
        "SQUARE_ADD_ANT",
        next_row + 1,
        Spec(
            body=Src0 * Src0 + Src1,
            reference=lambda in0, in1, s0, s1, imm2: (
                in0.astype(np.float32) ** 2 + in1
            ).astype(np.float32),
        ),
    )
    u_node = Src0 * Src0
    sig_op = make(
        "SIGMOID_POLY_ANT",
        next_row + 2,
        Spec(
            body=(((u_node * C2 + C1) * u_node + C0) * Src0) + Src1,
            reference=lambda in0, in1, s0, s1, imm2: (
                ((in0.astype(np.float32) ** 2 * imm2 + s1) * in0**2 + s0) * in0 + in1
            ).astype(np.float32),
        ),
    )
    dops._BY_NAME_ANT = {
        "SQUARE_PS_ANT": sq_op,
        "SQUARE_ADD_ANT": sqa_op,
        "SIGMOID_POLY_ANT": sig_op,
    }
    return sq_op, sqa_op, sig_op

B, H, W = 32, 512, 512
NCORES = 8
BPC = B // NCORES  # images per core

SQ2 = float(np.sqrt(2.0))

# (r0, M, s0, K, align): out rows [r0, r0+M), input tile = image rows [s0, s0+K)
CHUNKS = [
    (0, 103, 0, 104, "top"),
    (103, 103, 102, 105, "mid"),
    (206, 103, 205, 105, "mid"),
    (309, 103, 308, 105, "mid"),
    (412, 100, 411, 101, "mid"),
]

# vertical tap sets (t=0 <-> dr=-1): out[r] = sum_t taps[t] * x[r + t - 1]
TAPS = {
    "c2d": (2.0, 0.0, -2.0),  # 2*d   (f0 center column)
    "c1d": (1.0, 0.0, -1.0),  # d     (f0 side columns)
    "cqd": (SQ2, 0.0, -SQ2),  # sqrt2*d (g2, all columns)
    "cs": (1.0, 2.0, 1.0),  # s     (f1 on Dh / +xm)
    "cqb": (SQ2, SQ2, SQ2),  # sqrt2*b (g3 on Dh / +xm)
    "ncs": (-1.0, -2.0, -1.0),  # -s  (f1 on xp, v4)
    "ncqb": (-SQ2, -SQ2, -SQ2),  # -sqrt2*b (g3 on xp, v4)
}

# sigmoid(t) ~= 0.5 + t*(P_C1 + u*(P_C3 + u*P_C5)), u = t^2, t in [0, 1.04];
# minimax fit, max abs err 3.3e-6.
P_C1, P_C3, P_C5 = 0.24997775, -0.02066035, 0.0017408


def _band(K, M, taps, align):
    """Banded lhsT [K, M]: out[m] = sum_k V[k, m] * tile[k]."""
    V = np.zeros((K, M), np.float32)
    base = 0 if align == "top" else 1
    for m in range(M):
        for t in range(3):
            k = m + t - 1 + base
            if 0 <= k < K:
                V[k, m] = taps[t]
    return V


def _build_weights():
    """Pack all band matrices into one [128, total_cols] array.

    Returns (wts, offmap) with offmap[(align, K, M)][type_name] = column offset.
    """
    offmap = {}
    mats = []
    off = 0
    for r0, M, s0, K, align in CHUNKS:
        key = (align, K, M)
        if key in offmap:
            continue
        offmap[key] = {}
        for tn, taps in TAPS.items():
            offmap[key][tn] = off
            mats.append((off, _band(K, M, taps, align)))
            off += M
    wts = np.zeros((128, off), np.float32)
    for o, V in mats:
        wts[: V.shape[0], o : o + V.shape[1]] = V
    return wts, offmap


def _build_nc(wts_cols, offmap, repeat=1):
    sq_op = sqa_op = sig_op = None
    if KERNEL_V >= 2:
        sq_op, sqa_op, sig_op = _register_custom_ops()
    # v6: bf16 end-to-end (host casts x/y); halves DMA and 4x faster matmuls.
    DT = BF16 if KERNEL_V >= 6 else F32
    nc = bacc.Bacc()
    x = nc.dram_tensor("x", [BPC, H, W], DT, kind="ExternalInput")
    wt = nc.dram_tensor("wts", [128, wts_cols], DT, kind="ExternalInput")
    y = nc.dram_tensor("y", [BPC, H, W], DT, kind="ExternalOutput")

    with TileContext(nc) as tc:
        with (
            tc.tile_pool(name="wpool", bufs=1) as wpool,
            tc.tile_pool(name="xpool", bufs=3) as xpool,
            tc.tile_pool(name="dpool", bufs=2) as dpool,
            tc.tile_pool(name="sqpool", bufs=2) as sqpool,
            tc.tile_pool(
                name="rpool", bufs=(4 if KERNEL_V >= 6 else len(CHUNKS) * BPC)
            ) as rpool,
            tc.tile_pool(name="psum", bufs=2, space="PSUM") as psp,
        ):
            wtile = wpool.tile([128, wts_cols], DT)
            nc.sync.dma_start(out=wtile[:], in_=wt[:])
            halfs = None
            if KERNEL_V >= 3:
                halfs = wpool.tile([128, 512], DT, tag="halfs")
                nc.vector.memset(halfs[:], 0.5)

            tail = []  # (rt, M, img, r0)
            last_sqrt = None

            def front(img, r0, M, s0, K, align):
                """DMA-in, horizontal diff, matmuls (+ v1: wide squares)."""
                voff = offmap[(align, K, M)]
                xt = xpool.tile([128, 516], DT, tag="xt")
                nc.vector.memset(xt[:K, 1:2], 0.0)
                if KERNEL_V < 4:
                    nc.vector.memset(xt[:K, 514:515], 0.0)
                nc.sync.dma_start(out=xt[:K, 2:514], in_=x[img, s0 : s0 + K, :])

                xm = xt[:K, 1:513]
                xc = xt[:K, 2:514]
                xp = xt[:K, 3:515]

                def wv(tn):
                    o = voff[tn]
                    return wtile[0:K, o : o + M]

                ps01 = psp.tile([128, 1024], F32, tag="f01")
                ps23 = psp.tile([128, 1024], F32, tag="g23")

                if KERNEL_V >= 4:
                    # No Dh: f1/g3 via +/- matrices. Partial-width accumulating
                    # MMs trim the right image edge (col 511 reads x[512]=0) so
                    # only the left pad column (col index 1 in xt) is needed.
                    xpt = xt[:K, 3:514]  # x[w+1] for w in [0, 511)
                    nc.tensor.matmul(ps01[:M, 512:1024], wv("cs"), xm, start=True, stop=False)
                    nc.tensor.matmul(ps01[:M, 512:1023], wv("ncs"), xpt, start=False, stop=True)
                    nc.tensor.matmul(ps23[:M, 512:1024], wv("cqb"), xm, start=True, stop=False)
                    nc.tensor.matmul(ps23[:M, 512:1023], wv("ncqb"), xpt, start=False, stop=True)
                    nc.tensor.matmul(ps01[:M, 0:512], wv("c2d"), xc, start=True, stop=False)
                    nc.tensor.matmul(ps01[:M, 0:512], wv("c1d"), xm, start=False, stop=False)
                    nc.tensor.matmul(ps01[:M, 0:511], wv("c1d"), xpt, start=False, stop=True)
                    nc.tensor.matmul(ps23[:M, 0:512], wv("cqd"), xm, start=True, stop=False)
                    nc.tensor.matmul(ps23[:M, 0:512], wv("cqd"), xc, start=False, stop=False)
                    nc.tensor.matmul(ps23[:M, 0:511], wv("cqd"), xpt, start=False, stop=True)
                else:
                    dh = dpool.tile([128, 512], F32, tag="dh")
                    nc.vector.tensor_sub(out=dh[:K], in0=xm, in1=xp)
                    # dh-consuming MMs first: one DVE sem transitively covers
                    # the xt DMA + memsets (per-instruction sync-wait budget).
                    nc.tensor.matmul(ps01[:M, 512:1024], wv("cs"), dh[:K], start=True, stop=True)
                    nc.tensor.matmul(ps23[:M, 512:1024], wv("cqb"), dh[:K], start=True, stop=True)
                    nc.tensor.matmul(ps01[:M, 0:512], wv("c2d"), xc, start=True, stop=False)
                    nc.tensor.matmul(ps01[:M, 0:512], wv("c1d"), xm, start=False, stop=False)
                    nc.tensor.matmul(ps01[:M, 0:512], wv("c1d"), xp, start=False, stop=True)
                    nc.tensor.matmul(ps23[:M, 0:512], wv("cqd"), xm, start=True, stop=False)
                    nc.tensor.matmul(ps23[:M, 0:512], wv("cqd"), xc, start=False, stop=False)
                    nc.tensor.matmul(ps23[:M, 0:512], wv("cqd"), xp, start=False, stop=True)

                if KERNEL_V == 1:
                    a01 = sqpool.tile([128, 1024], F32, tag="a01")
                    nc.scalar.activation(a01[:M], ps01[:M, :], AF.Square)
                    a23 = sqpool.tile([128, 1024], F32, tag="a23")
                    nc.scalar.activation(a23[:M], ps23[:M, :], AF.Square)
                    return a01, a23
                if KERNEL_V == 5:
                    # Rebalance DVE->ACT: ScalarE squares the f0/f1 pair here
                    # (it is otherwise idle but for the sqrt), DVE keeps g2/g3.
                    a01 = sqpool.tile([128, 1024], F32, tag="a01")
                    nc.scalar.activation(a01[:M], ps01[:M, :], AF.Square)
                    return a01, ps23
                return ps01, ps23

            def back_v6(st, chunk_idx):
                """v6 tail: ACT squares ps01 wide -> bf16, DVE native 2x adds,
                custom sqa chain on ps23 (1x, PSUM), ACT sqrt, DVE sigmoid
                poly, bf16 out. Every ~3rd chunk shifts g2/g3 squaring to ACT
                (type-2) to balance DVE vs ACT occupancy."""
                (t01, t23), (img, r0, M) = st
                type2 = chunk_idx % 8 in (1, 4, 6)
                sq01 = sqpool.tile([128, 1024], BF16, tag="sq01")
                nc.scalar.activation(sq01[:M], t01[:M, :], AF.Square)
                A = sqpool.tile([128, 512], BF16, tag="A")
                nc.vector.tensor_add(
                    out=A[:M], in0=sq01[:M, 0:512], in1=sq01[:M, 512:1024]
                )
                ss = sqpool.tile([128, 512], BF16, tag="ss")
                if type2:
                    sq23 = sqpool.tile([128, 1024], BF16, tag="sq23")
                    nc.scalar.activation(sq23[:M], t23[:M, :], AF.Square)
                    Bt = sqpool.tile([128, 512], BF16, tag="Bt")
                    nc.vector.tensor_add(
                        out=Bt[:M], in0=sq23[:M, 0:512], in1=sq23[:M, 512:1024]
                    )
                    nc.vector.tensor_add(out=ss[:M], in0=A[:M], in1=Bt[:M])
                else:
                    c3 = sqpool.tile([128, 512], BF16, tag="c3")
                    nc.vector._custom_dve(
                        sqa_op, out=c3[:M], in0=t23[:M, 0:512], in1=A[:M]
                    )
                    nc.vector._custom_dve(
                        sqa_op, out=ss[:M], in0=t23[:M, 512:1024], in1=c3[:M]
                    )
                s = sqpool.tile([128, 512], BF16, tag="s")
                nc.scalar.activation(s[:M], ss[:M], AF.Sqrt, scale=1.0 / 64.0)
                rt = rpool.tile([128, 512], BF16, tag="rt")
                nc.vector._custom_dve(
                    sig_op,
                    out=rt[:M],
                    in0=s[:M],
                    in1=halfs[:M],
                    s0=P_C1,
                    s1=P_C3,
                    imm2=P_C5,
                )
                nc.sync.dma_start(out=y[img, r0 : r0 + M, :], in_=rt[:M])

            def back(st):
                """Sum of squares + sqrt (+sigmoid for v>=3), one chunk behind."""
                nonlocal last_sqrt
                (t01, t23), (img, r0, M) = st
                rt = rpool.tile([128, 512], F32, tag="rt")
                if KERNEL_V == 1:
                    u = sqpool.tile([128, 1024], F32, tag="u")
                    nc.vector.tensor_add(out=u[:M], in0=t01[:M], in1=t23[:M])
                    nc.vector.tensor_add(
                        out=rt[:M], in0=u[:M, 0:512], in1=u[:M, 512:1024]
                    )
                elif KERNEL_V == 5:
                    # t01 = ACT-squared f0/f1 pair (SBUF), t23 = raw g2/g3 PSUM.
                    s1 = sqpool.tile([128, 512], F32, tag="c1")
                    nc.vector.tensor_add(
                        out=s1[:M], in0=t01[:M, 0:512], in1=t01[:M, 512:1024]
                    )
                    c3 = sqpool.tile([128, 512], F32, tag="c3")
                    nc.vector._custom_dve(
                        sqa_op, out=c3[:M], in0=t23[:M, 0:512], in1=s1[:M]
                    )
                    nc.vector._custom_dve(
                        sqa_op, out=rt[:M], in0=t23[:M, 512:1024], in1=c3[:M]
                    )
                else:
                    # DVE custom chain: SS = f0^2 + f1^2 + g2^2 + g3^2 with one
                    # PSUM read per map and no ScalarE involvement.
                    c1 = sqpool.tile([128, 512], F32, tag="c1")
                    nc.vector._custom_dve(sq_op, out=c1[:M], in0=t01[:M, 0:512])
                    c2 = sqpool.tile([128, 512], F32, tag="c2")
                    nc.vector._custom_dve(
                        sqa_op, out=c2[:M], in0=t01[:M, 512:1024], in1=c1[:M]
                    )
                    c3 = sqpool.tile([128, 512], F32, tag="c3")
                    nc.vector._custom_dve(
                        sqa_op, out=c3[:M], in0=t23[:M, 0:512], in1=c2[:M]
                    )
                    nc.vector._custom_dve(
                        sqa_op, out=rt[:M], in0=t23[:M, 512:1024], in1=c3[:M]
                    )
                # v>=3: Sqrt(SS/64) = sqrt(SS)/8 exactly (power-of-two scale),
                # so the DVE sigmoid polynomial takes t directly.
                sqrt_scale = 1.0 / 64.0 if KERNEL_V >= 3 else 1.0
                last_sqrt = nc.scalar.activation(
                    rt[:M], rt[:M], AF.Sqrt, scale=sqrt_scale
                )
                if KERNEL_V >= 3:
                    # sigmoid(r/8) via deg-5 odd polynomial on DVE; scale 1/8
                    # folded into the ACT sqrt would disturb nothing, but the
                    # poly coefficients already absorb it via scale on sqrt.
                    nc.vector._custom_dve(
                        sig_op,
                        out=rt[:M],
                        in0=rt[:M],
                        in1=halfs[:M],
                        s0=P_C1,
                        s1=P_C3,
                        imm2=P_C5,
                    )
                    nc.sync.dma_start(out=y[img, r0 : r0 + M, :], in_=rt[:M])
                else:
                    tail.append((rt, M, img, r0))

            for rep in range(repeat):
                tail.clear()
                pending = None
                ci = 0
                for img in range(BPC):
                    for r0, M, s0, K, align in CHUNKS:
                        sq = front(img, r0, M, s0, K, align)
                        if pending is not None:
                            if KERNEL_V >= 6:
                                back_v6(pending, ci)
                            else:
                                back(pending)
                            ci += 1
                        pending = (sq, (img, r0, M))
                if KERNEL_V >= 6:
                    back_v6(pending, ci)
                else:
                    back(pending)

                # Batched sigmoids: forced after all sqrt work so only one ACT
                # table-set switch happens (sqrt_and_others -> sigmoid_and_others).
                prev = last_sqrt
                for rt, M, img, r0 in tail:
                    si = nc.scalar.activation(rt[:M], rt[:M], AF.Sigmoid, scale=0.125)
                    add_dep_helper(si.ins, prev.ins, False, "batch sigmoids after sqrts")
                    prev = si
                    nc.sync.dma_start(out=y[img, r0 : r0 + M, :], in_=rt[:M])

    nc.compile()
    return nc


_CACHE = {}


def _get_nc():
    global KERNEL_V
    KERNEL_V = int(os.environ.get("KERNEL_V", "5"))
    repeat = int(os.environ.get("KERNEL_REPEAT", "1"))
    key = ("nc", repeat, KERNEL_V)
    if key not in _CACHE:
        wts, offmap = _build_weights()
        _CACHE["wts"] = wts
        _CACHE[key] = _build_nc(wts.shape[1], offmap, repeat=repeat)
    return _CACHE[key], _CACHE["wts"]


def _get_nc_for_bench():
    """bench_ab hook: (nc, extra per-core inputs beyond x)."""
    nc, wts = _get_nc()
    if KERNEL_V >= 6:
        import ml_dtypes

        wts = wts.astype(ml_dtypes.bfloat16)
    return nc, {"wts": wts}


_last_result = None


def kernel(pred_mask: np.ndarray) -> np.ndarray:
    global _last_result
    from concourse.bass_utils import run_bass_kernel_spmd

    assert pred_mask.shape == (B, 1, H, W), pred_mask.shape
    nc, wts = _get_nc()
    if KERNEL_V >= 6:
        import ml_dtypes

        xs = np.ascontiguousarray(
            pred_mask.reshape(B, H, W).astype(ml_dtypes.bfloat16)
        )
        wts = wts.astype(ml_dtypes.bfloat16)
    else:
        xs = np.ascontiguousarray(pred_mask.reshape(B, H, W).astype(np.float32))
    in_maps = [
        {"x": xs[i * BPC : (i + 1) * BPC], "wts": wts} for i in range(NCORES)
    ]
    trace = bool(os.environ.get("KERNEL_TRACE"))
    res = run_bass_kernel_spmd(
        nc, in_maps, core_ids=list(range(NCORES)), trace=trace
    )
    _last_result = res
    out = np.stack([r["y"] for r in res.results], axis=0)
    return out.reshape(B, 1, H, W).astype(np.float32)



# revision 13
# speedup vs baseline: 3.3557x; 3.3557x over previous
"""Boundary-calculation module (4 fixed 3x3 Sobel-like kernels -> sqrt-sum-sq -> sigmoid)
as a Trainium2 Bass kernel, data-parallel over 8 NeuronCores (batch 32 -> 4 images/core).

Math: with integer kernels K_k (reference kernels x4), the output is
    out = sigmoid(sqrt(SS)/8),  SS = E0^2 + E1^2 + E2^2 + E3^2
and the filter bank is rotated into four *separable* filters (exact identity):
    SS = f0^2 + f1^2 + g2^2 + g3^2
    f0 = diffv(smoothh(x))          (= E0)
    f1 = smoothv(diffh(x))          (= E1)
    g2 = sqrt(2)*diffv(boxh(x))
    g3 = sqrt(2)*boxv(diffh(x))
Vertical 3-tap convs run on the TensorEngine as banded matmuls (lhsT = band
matrix); horizontal shifts are free rhs access-pattern offsets; diffh runs once
on the VectorEngine. Squares/sums on ACT+DVE, sqrt on ACT (sqrt_and_others
table set), sigmoids batched at the end (sigmoid_and_others, single switch).
"""

import os
import sys

sys.path.insert(0, "/opt/trn_rl_repo")

import numpy as np

import concourse.bacc as bacc
import concourse.bass as bass
import concourse.mybir as mybir
from concourse.tile import TileContext, add_dep_helper

AF = mybir.ActivationFunctionType
F32 = mybir.dt.float32
BF16 = mybir.dt.bfloat16

KERNEL_V = int(os.environ.get("KERNEL_V", "6"))


def _register_custom_ops():
    """Register SQUARE_PS_ANT (out = in0^2) and SQUARE_ADD_ANT
    (out = in0^2 + in1) as custom DVE ops at runtime. They let the DVE read
    each PSUM map once and produce the running sum-of-squares without any
    ScalarE Square passes."""
    import concourse.dve_ops as dops
    from concourse.dve_spec import Spec, Src0, Src1, C0, C1, C2, lower, _has_src1
    from concourse.dve_uop import DveOpSpec

    if "SQUARE_PS_ANT" in dops._SUB_OPCODE_FOR_NAME:
        return (
            dops._BY_NAME_ANT["SQUARE_PS_ANT"],
            dops._BY_NAME_ANT["SQUARE_ADD_ANT"],
            dops._BY_NAME_ANT["SIGMOID_POLY_ANT"],
        )

    def make(name, row, spec):
        dops._SUB_OPCODE_FOR_NAME[name] = row
        shas = {}
        for ver in ("v3", "v4"):
            try:
                compiled = DveOpSpec(
                    name=name,
                    opcode=row,
                    uops=lower(spec, ver=ver),
                    rd1_en=_has_src1(spec),
                )
                shas[ver] = compiled.sha(ver)
            except Exception:
                pass
        op = dops.DveOp(name, spec, False, shas)
        dops.OPS.append(op)
        dops.CUSTOM_DVE_SPECS[name] = spec
        return op

    next_row = max(dops._SUB_OPCODE_FOR_NAME.values()) + 1
    # NB: sq(Src0) lowers to something the DVE firmware rejects
    # (NRT_EXEC_UNIT_UNRECOVERABLE on HW); Src0*Src0 works.
    sq_op = make(
        "SQUARE_PS_ANT",
        next_row,
        Spec(
            body=Src0 * Src0,
            reference=lambda in0, in1, s0, s1, imm2: (
                in0.astype(np.float32) ** 2
            ).astype(np.float32),
        ),
    )
    sqa_op = make(
        "SQUARE_ADD_ANT",
        next_row + 1,
        Spec(
            body=Src0 * Src0 + Src1,
            reference=lambda in0, in1, s0, s1, imm2: (
                in0.astype(np.float32) ** 2 + in1
            ).astype(np.float32),
        ),
    )
    u_node = Src0 * Src0
    sig_op = make(
        "SIGMOID_POLY_ANT",
        next_row + 2,
        Spec(
            body=(((u_node * C2 + C1) * u_node + C0) * Src0) + Src1,
            reference=lambda in0, in1, s0, s1, imm2: (
                ((in0.astype(np.float32) ** 2 * imm2 + s1) * in0**2 + s0) * in0 + in1
            ).astype(np.float32),
        ),
    )
    dops._BY_NAME_ANT = {
        "SQUARE_PS_ANT": sq_op,
        "SQUARE_ADD_ANT": sqa_op,
        "SIGMOID_POLY_ANT": sig_op,
    }
    return sq_op, sqa_op, sig_op

B, H, W = 32, 512, 512
NCORES = 8
BPC = B // NCORES  # images per core

SQ2 = float(np.sqrt(2.0))

# (r0, M, s0, K, align): out rows [r0, r0+M), input tile = image rows [s0, s0+K)
CHUNKS = [
    (0, 103, 0, 104, "top"),
    (103, 103, 102, 105, "mid"),
    (206, 103, 205, 105, "mid"),
    (309, 103, 308, 105, "mid"),
    (412, 100, 411, 101, "mid"),
]

# vertical tap sets (t=0 <-> dr=-1): out[r] = sum_t taps[t] * x[r + t - 1]
TAPS = {
    "c2d": (2.0, 0.0, -2.0),  # 2*d   (f0 center column)
    "c1d": (1.0, 0.0, -1.0),  # d     (f0 side columns)
    "cqd": (SQ2, 0.0, -SQ2),  # sqrt2*d (g2, all columns)
    "cs": (1.0, 2.0, 1.0),  # s     (f1 on Dh / +xm)
    "cqb": (SQ2, SQ2, SQ2),  # sqrt2*b (g3 on Dh / +xm)
    "ncs": (-1.0, -2.0, -1.0),  # -s  (f1 on xp, v4)
    "ncqb": (-SQ2, -SQ2, -SQ2),  # -sqrt2*b (g3 on xp, v4)
}

# sigmoid(t) ~= 0.5 + t*(P_C1 + u*(P_C3 + u*P_C5)), u = t^2, t in [0, 1.04];
# minimax fit, max abs err 3.3e-6.
P_C1, P_C3, P_C5 = 0.24997775, -0.02066035, 0.0017408


def _band(K, M, taps, align):
    """Banded lhsT [K, M]: out[m] = sum_k V[k, m] * tile[k]."""
    V = np.zeros((K, M), np.float32)
    base = 0 if align == "top" else 1
    for m in range(M):
        for t in range(3):
            k = m + t - 1 + base
            if 0 <= k < K:
                V[k, m] = taps[t]
    return V


def _build_weights():
    """Pack all band matrices into one [128, total_cols] array.

    Returns (wts, offmap) with offmap[(align, K, M)][type_name] = column offset.
    """
    offmap = {}
    mats = []
    off = 0
    for r0, M, s0, K, align in CHUNKS:
        key = (align, K, M)
        if key in offmap:
            continue
        offmap[key] = {}
        for tn, taps in TAPS.items():
            offmap[key][tn] = off
            mats.append((off, _band(K, M, taps, align)))
            off += M
    wts = np.zeros((128, off), np.float32)
    for o, V in mats:
        wts[: V.shape[0], o : o + V.shape[1]] = V
    return wts, offmap


def _build_nc(wts_cols, offmap, repeat=1):
    sq_op = sqa_op = sig_op = None
    if KERNEL_V >= 2:
        sq_op, sqa_op, sig_op = _register_custom_ops()
    # v6: bf16 end-to-end (host casts x/y); halves DMA and 4x faster matmuls.
    DT = BF16 if KERNEL_V >= 6 else F32
    nc = bacc.Bacc()
    x = nc.dram_tensor("x", [BPC, H, W], DT, kind="ExternalInput")
    wt = nc.dram_tensor("wts", [128, wts_cols], DT, kind="ExternalInput")
    y = nc.dram_tensor("y", [BPC, H, W], DT, kind="ExternalOutput")

    with TileContext(nc) as tc:
        with (
            tc.tile_pool(name="wpool", bufs=1) as wpool,
            tc.tile_pool(name="xpool", bufs=3) as xpool,
            tc.tile_pool(name="dpool", bufs=2) as dpool,
            tc.tile_pool(name="sqpool", bufs=2) as sqpool,
            tc.tile_pool(
                name="rpool", bufs=(4 if KERNEL_V >= 6 else len(CHUNKS) * BPC)
            ) as rpool,
            tc.tile_pool(name="psum", bufs=2, space="PSUM") as psp,
        ):
            wtile = wpool.tile([128, wts_cols], DT)
            nc.sync.dma_start(out=wtile[:], in_=wt[:])
            halfs = None
            if KERNEL_V >= 3:
                halfs = wpool.tile([128, 512], DT, tag="halfs")
                nc.vector.memset(halfs[:], 0.5)

            tail = []  # (rt, M, img, r0)
            last_sqrt = None

            def front(img, r0, M, s0, K, align):
                """DMA-in, horizontal diff, matmuls (+ v1: wide squares)."""
                voff = offmap[(align, K, M)]
                xt = xpool.tile([128, 516], DT, tag="xt")
                nc.vector.memset(xt[:K, 1:2], 0.0)
                if KERNEL_V < 4:
                    nc.vector.memset(xt[:K, 514:515], 0.0)
                nc.sync.dma_start(out=xt[:K, 2:514], in_=x[img, s0 : s0 + K, :])

                xm = xt[:K, 1:513]
                xc = xt[:K, 2:514]
                xp = xt[:K, 3:515]

                def wv(tn):
                    o = voff[tn]
                    return wtile[0:K, o : o + M]

                ps01 = psp.tile([128, 1024], F32, tag="f01")
                ps23 = psp.tile([128, 1024], F32, tag="g23")

                if KERNEL_V >= 4:
                    # No Dh: f1/g3 via +/- matrices. Partial-width accumulating
                    # MMs trim the right image edge (col 511 reads x[512]=0) so
                    # only the left pad column (col index 1 in xt) is needed.
                    xpt = xt[:K, 3:514]  # x[w+1] for w in [0, 511)
                    nc.tensor.matmul(ps01[:M, 512:1024], wv("cs"), xm, start=True, stop=False)
                    nc.tensor.matmul(ps01[:M, 512:1023], wv("ncs"), xpt, start=False, stop=True)
                    nc.tensor.matmul(ps23[:M, 512:1024], wv("cqb"), xm, start=True, stop=False)
                    nc.tensor.matmul(ps23[:M, 512:1023], wv("ncqb"), xpt, start=False, stop=True)
                    nc.tensor.matmul(ps01[:M, 0:512], wv("c2d"), xc, start=True, stop=False)
                    nc.tensor.matmul(ps01[:M, 0:512], wv("c1d"), xm, start=False, stop=False)
                    nc.tensor.matmul(ps01[:M, 0:511], wv("c1d"), xpt, start=False, stop=True)
                    nc.tensor.matmul(ps23[:M, 0:512], wv("cqd"), xm, start=True, stop=False)
                    nc.tensor.matmul(ps23[:M, 0:512], wv("cqd"), xc, start=False, stop=False)
                    nc.tensor.matmul(ps23[:M, 0:511], wv("cqd"), xpt, start=False, stop=True)
                else:
                    dh = dpool.tile([128, 512], F32, tag="dh")
                    nc.vector.tensor_sub(out=dh[:K], in0=xm, in1=xp)
                    # dh-consuming MMs first: one DVE sem transitively covers
                    # the xt DMA + memsets (per-instruction sync-wait budget).
                    nc.tensor.matmul(ps01[:M, 512:1024], wv("cs"), dh[:K], start=True, stop=True)
                    nc.tensor.matmul(ps23[:M, 512:1024], wv("cqb"), dh[:K], start=True, stop=True)
                    nc.tensor.matmul(ps01[:M, 0:512], wv("c2d"), xc, start=True, stop=False)
                    nc.tensor.matmul(ps01[:M, 0:512], wv("c1d"), xm, start=False, stop=False)
                    nc.tensor.matmul(ps01[:M, 0:512], wv("c1d"), xp, start=False, stop=True)
                    nc.tensor.matmul(ps23[:M, 0:512], wv("cqd"), xm, start=True, stop=False)
                    nc.tensor.matmul(ps23[:M, 0:512], wv("cqd"), xc, start=False, stop=False)
                    nc.tensor.matmul(ps23[:M, 0:512], wv("cqd"), xp, start=False, stop=True)

                if KERNEL_V == 1:
                    a01 = sqpool.tile([128, 1024], F32, tag="a01")
                    nc.scalar.activation(a01[:M], ps01[:M, :], AF.Square)
                    a23 = sqpool.tile([128, 1024], F32, tag="a23")
                    nc.scalar.activation(a23[:M], ps23[:M, :], AF.Square)
                    return a01, a23
                if KERNEL_V == 5:
                    # Rebalance DVE->ACT: ScalarE squares the f0/f1 pair here
                    # (it is otherwise idle but for the sqrt), DVE keeps g2/g3.
                    a01 = sqpool.tile([128, 1024], F32, tag="a01")
                    nc.scalar.activation(a01[:M], ps01[:M, :], AF.Square)
                    return a01, ps23
                return ps01, ps23

            def back_v6(st, chunk_idx):
                """v6 tail: ACT squares ps01 wide -> bf16, DVE native 2x adds,
                custom sqa chain on ps23 (1x, PSUM), ACT sqrt, DVE sigmoid
                poly, bf16 out. Every ~3rd chunk shifts g2/g3 squaring to ACT
                (type-2) to balance DVE vs ACT occupancy."""
                (t01, t23), (img, r0, M) = st
                type2 = chunk_idx % 8 in (1, 4, 6)
                sq01 = sqpool.tile([128, 1024], BF16, tag="sq01")
                nc.scalar.activation(sq01[:M], t01[:M, :], AF.Square)
                A = sqpool.tile([128, 512], BF16, tag="A")
                nc.vector.tensor_add(
                    out=A[:M], in0=sq01[:M, 0:512], in1=sq01[:M, 512:1024]
                )
                ss = sqpool.tile([128, 512], BF16, tag="ss")
                if type2:
                    sq23 = sqpool.tile([128, 1024], BF16, tag="sq23")
                    nc.scalar.activation(sq23[:M], t23[:M, :], AF.Square)
                    Bt = sqpool.tile([128, 512], BF16, tag="Bt")
                    nc.vector.tensor_add(
                        out=Bt[:M], in0=sq23[:M, 0:512], in1=sq23[:M, 512:1024]
                    )
                    nc.vector.tensor_add(out=ss[:M], in0=A[:M], in1=Bt[:M])
                else:
                    c3 = sqpool.tile([128, 512], BF16, tag="c3")
                    nc.vector._custom_dve(
                        sqa_op, out=c3[:M], in0=t23[:M, 0:512], in1=A[:M]
                    )
                    nc.vector._custom_dve(
                        sqa_op, out=ss[:M], in0=t23[:M, 512:1024], in1=c3[:M]
                    )
                s = sqpool.tile([128, 512], BF16, tag="s")
                nc.scalar.activation(s[:M], ss[:M], AF.Sqrt, scale=1.0 / 64.0)
                rt = rpool.tile([128, 512], BF16, tag="rt")
                nc.vector._custom_dve(
                    sig_op,
                    out=rt[:M],
                    in0=s[:M],
                    in1=halfs[:M],
                    s0=P_C1,
                    s1=P_C3,
                    imm2=P_C5,
                )
                nc.sync.dma_start(out=y[img, r0 : r0 + M, :], in_=rt[:M])

            def back(st):
                """Sum of squares + sqrt (+sigmoid for v>=3), one chunk behind."""
                nonlocal last_sqrt
                (t01, t23), (img, r0, M) = st
                rt = rpool.tile([128, 512], F32, tag="rt")
                if KERNEL_V == 1:
                    u = sqpool.tile([128, 1024], F32, tag="u")
                    nc.vector.tensor_add(out=u[:M], in0=t01[:M], in1=t23[:M])
                    nc.vector.tensor_add(
                        out=rt[:M], in0=u[:M, 0:512], in1=u[:M, 512:1024]
                    )
                elif KERNEL_V == 5:
                    # t01 = ACT-squared f0/f1 pair (SBUF), t23 = raw g2/g3 PSUM.
                    s1 = sqpool.tile([128, 512], F32, tag="c1")
                    nc.vector.tensor_add(
                        out=s1[:M], in0=t01[:M, 0:512], in1=t01[:M, 512:1024]
                    )
                    c3 = sqpool.tile([128, 512], F32, tag="c3")
                    nc.vector._custom_dve(
                        sqa_op, out=c3[:M], in0=t23[:M, 0:512], in1=s1[:M]
                    )
                    nc.vector._custom_dve(
                        sqa_op, out=rt[:M], in0=t23[:M, 512:1024], in1=c3[:M]
                    )
                else:
                    # DVE custom chain: SS = f0^2 + f1^2 + g2^2 + g3^2 with one
                    # PSUM read per map and no ScalarE involvement.
                    c1 = sqpool.tile([128, 512], F32, tag="c1")
                    nc.vector._custom_dve(sq_op, out=c1[:M], in0=t01[:M, 0:512])
                    c2 = sqpool.tile([128, 512], F32, tag="c2")
                    nc.vector._custom_dve(
                        sqa_op, out=c2[:M], in0=t01[:M, 512:1024], in1=c1[:M]
                    )
                    c3 = sqpool.tile([128, 512], F32, tag="c3")
                    nc.vector._custom_dve(
                        sqa_op, out=c3[:M], in0=t23[:M, 0:512], in1=c2[:M]
                    )
                    nc.vector._custom_dve(
                        sqa_op, out=rt[:M], in0=t23[:M, 512:1024], in1=c3[:M]
                    )
                # v>=3: Sqrt(SS/64) = sqrt(SS)/8 exactly (power-of-two scale),
                # so the DVE sigmoid polynomial takes t directly.
                sqrt_scale = 1.0 / 64.0 if KERNEL_V >= 3 else 1.0
                last_sqrt = nc.scalar.activation(
                    rt[:M], rt[:M], AF.Sqrt, scale=sqrt_scale
                )
                if KERNEL_V >= 3:
                    # sigmoid(r/8) via deg-5 odd polynomial on DVE; scale 1/8
                    # folded into the ACT sqrt would disturb nothing, but the
                    # poly coefficients already absorb it via scale on sqrt.
                    nc.vector._custom_dve(
                        sig_op,
                        out=rt[:M],
                        in0=rt[:M],
                        in1=halfs[:M],
                        s0=P_C1,
                        s1=P_C3,
                        imm2=P_C5,
                    )
                    nc.sync.dma_start(out=y[img, r0 : r0 + M, :], in_=rt[:M])
                else:
                    tail.append((rt, M, img, r0))

            for rep in range(repeat):
                tail.clear()
                pending = None
                ci = 0
                for img in range(BPC):
                    for r0, M, s0, K, align in CHUNKS:
                        sq = front(img, r0, M, s0, K, align)
                        if pending is not None:
                            if KERNEL_V >= 6:
                                back_v6(pending, ci)
                            else:
                                back(pending)
                            ci += 1
                        pending = (sq, (img, r0, M))
                if KERNEL_V >= 6:
                    back_v6(pending, ci)
                else:
                    back(pending)

                # Batched sigmoids: forced after all sqrt work so only one ACT
                # table-set switch happens (sqrt_and_others -> sigmoid_and_others).
                prev = last_sqrt
                for rt, M, img, r0 in tail:
                    si = nc.scalar.activation(rt[:M], rt[:M], AF.Sigmoid, scale=0.125)
                    add_dep_helper(si.ins, prev.ins, False, "batch sigmoids after sqrts")
                    prev = si
                    nc.sync.dma_start(out=y[img, r0 : r0 + M, :], in_=rt[:M])

    nc.compile()
    return nc


_CACHE = {}


def _get_nc():
    global KERNEL_V
    KERNEL_V = int(os.environ.get("KERNEL_V", "6"))
    repeat = int(os.environ.get("KERNEL_REPEAT", "1"))
    key = ("nc", repeat, KERNEL_V)
    if key not in _CACHE:
        wts, offmap = _build_weights()
        _CACHE["wts"] = wts
        _CACHE[key] = _build_nc(wts.shape[1], offmap, repeat=repeat)
    return _CACHE[key], _CACHE["wts"]


def _get_nc_for_bench():
    """bench_ab hook: (nc, extra per-core inputs beyond x)."""
    nc, wts = _get_nc()
    if KERNEL_V >= 6:
        import ml_dtypes

        wts = wts.astype(ml_dtypes.bfloat16)
    return nc, {"wts": wts}


_last_result = None


def kernel(pred_mask: np.ndarray) -> np.ndarray:
    global _last_result
    from concourse.bass_utils import run_bass_kernel_spmd

    assert pred_mask.shape == (B, 1, H, W), pred_mask.shape
    nc, wts = _get_nc()
    if KERNEL_V >= 6:
        import ml_dtypes

        xs = np.ascontiguousarray(
            pred_mask.reshape(B, H, W).astype(ml_dtypes.bfloat16)
        )
        wts = wts.astype(ml_dtypes.bfloat16)
    else:
        xs = np.ascontiguousarray(pred_mask.reshape(B, H, W).astype(np.float32))
    in_maps = [
        {"x": xs[i * BPC : (i + 1) * BPC], "wts": wts} for i in range(NCORES)
    ]
    trace = bool(os.environ.get("KERNEL_TRACE"))
    res = run_bass_kernel_spmd(
        nc, in_maps, core_ids=list(range(NCORES)), trace=trace
    )
    _last_result = res
    out = np.stack([r["y"] for r in res.results], axis=0)
    return out.reshape(B, 1, H, W).astype(np.float32)



# revision 24
# speedup vs baseline: 3.7357x; 1.1133x over previous
"""Boundary-calculation module (4 fixed 3x3 Sobel-like kernels -> sqrt-sum-sq -> sigmoid)
as a Trainium2 Bass kernel, data-parallel over 8 NeuronCores (batch 32 -> 4 images/core).

Math: with integer kernels K_k (reference kernels x4), the output is
    out = sigmoid(sqrt(SS)/8),  SS = E0^2 + E1^2 + E2^2 + E3^2
and the filter bank is rotated into four *separable* filters (exact identity):
    SS = f0^2 + f1^2 + g2^2 + g3^2
    f0 = diffv(smoothh(x))          (= E0)
    f1 = smoothv(diffh(x))          (= E1)
    g2 = sqrt(2)*diffv(boxh(x))
    g3 = sqrt(2)*boxv(diffh(x))
Vertical 3-tap convs run on the TensorEngine as banded matmuls (lhsT = band
matrix); horizontal shifts are free rhs access-pattern offsets; diffh runs once
on the VectorEngine. Squares/sums on ACT+DVE, sqrt on ACT (sqrt_and_others
table set), sigmoids batched at the end (sigmoid_and_others, single switch).
"""

import os
import sys

sys.path.insert(0, "/opt/trn_rl_repo")

import numpy as np

import concourse.bacc as bacc
import concourse.bass as bass
import concourse.mybir as mybir
from concourse.tile import TileContext, add_dep_helper

AF = mybir.ActivationFunctionType
F32 = mybir.dt.float32
BF16 = mybir.dt.bfloat16

KERNEL_V = int(os.environ.get("KERNEL_V", "6"))
# Out of every 8 chunks, how many use the ACT-heavy type-2 tail (v6+).
KERNEL_MIX8 = int(os.environ.get("KERNEL_MIX8", "3"))


def _register_custom_ops():
    """Register SQUARE_PS_ANT (out = in0^2) and SQUARE_ADD_ANT
    (out = in0^2 + in1) as custom DVE ops at runtime. They let the DVE read
    each PSUM map once and produce the running sum-of-squares without any
    ScalarE Square passes."""
    import concourse.dve_ops as dops
    from concourse.dve_spec import Spec, Src0, Src1, C0, C1, C2, lower, _has_src1
    from concourse.dve_uop import DveOpSpec

    if "SQUARE_PS_ANT" in dops._SUB_OPCODE_FOR_NAME:
        return (
            dops._BY_NAME_ANT["SQUARE_PS_ANT"],
            dops._BY_NAME_ANT["SQUARE_ADD_ANT"],
            dops._BY_NAME_ANT["SIGMOID_POLY_ANT"],
        )

    def make(name, row, spec):
        dops._SUB_OPCODE_FOR_NAME[name] = row
        shas = {}
        for ver in ("v3", "v4"):
            try:
                compiled = DveOpSpec(
                    name=name,
                    opcode=row,
                    uops=lower(spec, ver=ver),
                    rd1_en=_has_src1(spec),
                )
                shas[ver] = compiled.sha(ver)
            except Exception:
                pass
        op = dops.DveOp(name, spec, False, shas)
        dops.OPS.append(op)
        dops.CUSTOM_DVE_SPECS[name] = spec
        return op

    next_row = max(dops._SUB_OPCODE_FOR_NAME.values()) + 1
    # NB: sq(Src0) lowers to something the DVE firmware rejects
    # (NRT_EXEC_UNIT_UNRECOVERABLE on HW); Src0*Src0 works.
    sq_op = make(
        "SQUARE_PS_ANT",
        next_row,
        Spec(
            body=Src0 * Src0,
            reference=lambda in0, in1, s0, s1, imm2: (
                in0.astype(np.float32) ** 2
            ).astype(np.float32),
        ),
    )
    sqa_op = make(
        "SQUARE_ADD_ANT",
        next_row + 1,
        Spec(
            body=Src0 * Src0 + Src1,
            reference=lambda in0, in1, s0, s1, imm2: (
                in0.astype(np.float32) ** 2 + in1
            ).astype(np.float32),
        ),
    )
    u_node = Src0 * Src0
    sig_op = make(
        "SIGMOID_POLY_ANT",
        next_row + 2,
        Spec(
            body=(((u_node * C2 + C1) * u_node + C0) * Src0) + Src1,
            reference=lambda in0, in1, s0, s1, imm2: (
                ((in0.astype(np.float32) ** 2 * imm2 + s1) * in0**2 + s0) * in0 + in1
            ).astype(np.float32),
        ),
    )
    dops._BY_NAME_ANT = {
        "SQUARE_PS_ANT": sq_op,
        "SQUARE_ADD_ANT": sqa_op,
        "SIGMOID_POLY_ANT": sig_op,
    }
    return sq_op, sqa_op, sig_op

B, H, W = 32, 512, 512
NCORES = 8
BPC = B // NCORES  # images per core

SQ2 = float(np.sqrt(2.0))

# (r0, M, s0, K, align): out rows [r0, r0+M), input tile = image rows [s0, s0+K)
CHUNKS = [
    (0, 103, 0, 104, "top"),
    (103, 103, 102, 105, "mid"),
    (206, 103, 205, 105, "mid"),
    (309, 103, 308, 105, "mid"),
    (412, 100, 411, 101, "mid"),
]

# vertical tap sets (t=0 <-> dr=-1): out[r] = sum_t taps[t] * x[r + t - 1]
TAPS = {
    "c2d": (2.0, 0.0, -2.0),  # 2*d   (f0 center column)
    "c1d": (1.0, 0.0, -1.0),  # d     (f0 side columns)
    "cqd": (SQ2, 0.0, -SQ2),  # sqrt2*d (g2, all columns)
    "cs": (1.0, 2.0, 1.0),  # s     (f1 on Dh / +xm)
    "cqb": (SQ2, SQ2, SQ2),  # sqrt2*b (g3 on Dh / +xm)
    "ncs": (-1.0, -2.0, -1.0),  # -s  (f1 on xp, v4)
    "ncqb": (-SQ2, -SQ2, -SQ2),  # -sqrt2*b (g3 on xp, v4)
}

# sigmoid(t) ~= 0.5 + t*(P_C1 + u*(P_C3 + u*P_C5)), u = t^2, t in [0, 1.04];
# minimax fit, max abs err 3.3e-6.
P_C1, P_C3, P_C5 = 0.24997775, -0.02066035, 0.0017408


def _band(K, M, taps, align):
    """Banded lhsT [K, M]: out[m] = sum_k V[k, m] * tile[k]."""
    V = np.zeros((K, M), np.float32)
    base = 0 if align == "top" else 1
    for m in range(M):
        for t in range(3):
            k = m + t - 1 + base
            if 0 <= k < K:
                V[k, m] = taps[t]
    return V


def _build_weights(pad128=False):
    """Pack all band matrices into one [128, total_cols] array.

    Returns (wts, offmap) with offmap[(align, K, M)][type_name] = column offset.
    pad128: stride each band to 128 columns (zero-padded) so matmuls can use
    full-width lhsT, enabling the compiler's fast-weight-load path.
    """
    offmap = {}
    mats = []
    off = 0
    for r0, M, s0, K, align in CHUNKS:
        key = (align, K, M)
        if key in offmap:
            continue
        offmap[key] = {}
        for tn, taps in TAPS.items():
            offmap[key][tn] = off
            mats.append((off, _band(K, M, taps, align)))
            off += 128 if pad128 else M
    wts = np.zeros((128, off), np.float32)
    for o, V in mats:
        wts[: V.shape[0], o : o + V.shape[1]] = V
    return wts, offmap


def _build_nc(wts_cols, offmap, repeat=1):
    sq_op = sqa_op = sig_op = None
    if KERNEL_V >= 2:
        sq_op, sqa_op, sig_op = _register_custom_ops()
    # v6: bf16 end-to-end (host casts x/y); halves DMA and 4x faster matmuls.
    DT = BF16 if KERNEL_V >= 6 else F32
    nc = bacc.Bacc()
    x = nc.dram_tensor("x", [BPC, H, W], DT, kind="ExternalInput")
    wt = nc.dram_tensor("wts", [128, wts_cols], DT, kind="ExternalInput")
    y = nc.dram_tensor("y", [BPC, H, W], DT, kind="ExternalOutput")

    with TileContext(nc) as tc:
        with (
            tc.tile_pool(name="wpool", bufs=1) as wpool,
            tc.tile_pool(name="xpool", bufs=3) as xpool,
            tc.tile_pool(name="dpool", bufs=2) as dpool,
            tc.tile_pool(name="sqpool", bufs=(4 if KERNEL_V >= 6 else 2)) as sqpool,
            tc.tile_pool(
                name="rpool", bufs=(4 if KERNEL_V >= 6 else len(CHUNKS) * BPC)
            ) as rpool,
            tc.tile_pool(name="psum", bufs=2, space="PSUM") as psp,
        ):
            wtile = wpool.tile([128, wts_cols], DT)
            nc.sync.dma_start(out=wtile[:], in_=wt[:])
            halfs = halfs2 = None
            if KERNEL_V >= 3:
                halfs = wpool.tile([128, 512], DT, tag="halfs")
                nc.vector.memset(halfs[:], 0.5)
            if KERNEL_V >= 7:
                halfs2 = wpool.tile([128, 1024], DT, tag="halfs2")
                nc.vector.memset(halfs2[:], 0.5)

            tail = []  # (rt, M, img, r0)
            last_sqrt = None

            def front(img, r0, M, s0, K, align):
                """DMA-in, horizontal diff, matmuls (+ v1: wide squares)."""
                voff = offmap[(align, K, M)]
                xt = xpool.tile([128, 516], DT, tag="xt")
                nc.vector.memset(xt[:K, 1:2], 0.0)
                if KERNEL_V < 4:
                    nc.vector.memset(xt[:K, 514:515], 0.0)
                nc.sync.dma_start(out=xt[:K, 2:514], in_=x[img, s0 : s0 + K, :])

                xm = xt[:K, 1:513]
                xc = xt[:K, 2:514]
                xp = xt[:K, 3:515]

                # v7: full-width lhsT (128 cols, zero-padded bands) for FWL;
                # psum rows M..128 get zeros-only contributions, never read.
                WM = 128 if KERNEL_V >= 7 else M
                PM = 128 if KERNEL_V >= 7 else M

                def wv(tn):
                    o = voff[tn]
                    return wtile[0:K, o : o + WM]

                ps01 = psp.tile([128, 1024], F32, tag="f01")
                ps23 = psp.tile([128, 1024], F32, tag="g23")

                if KERNEL_V >= 4:
                    # No Dh: f1/g3 via +/- matrices. Partial-width accumulating
                    # MMs trim the right image edge (col 511 reads x[512]=0) so
                    # only the left pad column (col index 1 in xt) is needed.
                    xpt = xt[:K, 3:514]  # x[w+1] for w in [0, 511)
                    nc.tensor.matmul(ps01[:PM, 512:1024], wv("cs"), xm, start=True, stop=False)
                    nc.tensor.matmul(ps01[:PM, 512:1023], wv("ncs"), xpt, start=False, stop=True)
                    nc.tensor.matmul(ps23[:PM, 512:1024], wv("cqb"), xm, start=True, stop=False)
                    nc.tensor.matmul(ps23[:PM, 512:1023], wv("ncqb"), xpt, start=False, stop=True)
                    nc.tensor.matmul(ps01[:PM, 0:512], wv("c2d"), xc, start=True, stop=False)
                    nc.tensor.matmul(ps01[:PM, 0:512], wv("c1d"), xm, start=False, stop=False)
                    nc.tensor.matmul(ps01[:PM, 0:511], wv("c1d"), xpt, start=False, stop=True)
                    nc.tensor.matmul(ps23[:PM, 0:512], wv("cqd"), xm, start=True, stop=False)
                    nc.tensor.matmul(ps23[:PM, 0:512], wv("cqd"), xc, start=False, stop=False)
                    nc.tensor.matmul(ps23[:PM, 0:511], wv("cqd"), xpt, start=False, stop=True)
                else:
                    dh = dpool.tile([128, 512], F32, tag="dh")
                    nc.vector.tensor_sub(out=dh[:K], in0=xm, in1=xp)
                    # dh-consuming MMs first: one DVE sem transitively covers
                    # the xt DMA + memsets (per-instruction sync-wait budget).
                    nc.tensor.matmul(ps01[:M, 512:1024], wv("cs"), dh[:K], start=True, stop=True)
                    nc.tensor.matmul(ps23[:M, 512:1024], wv("cqb"), dh[:K], start=True, stop=True)
                    nc.tensor.matmul(ps01[:M, 0:512], wv("c2d"), xc, start=True, stop=False)
                    nc.tensor.matmul(ps01[:M, 0:512], wv("c1d"), xm, start=False, stop=False)
                    nc.tensor.matmul(ps01[:M, 0:512], wv("c1d"), xp, start=False, stop=True)
                    nc.tensor.matmul(ps23[:M, 0:512], wv("cqd"), xm, start=True, stop=False)
                    nc.tensor.matmul(ps23[:M, 0:512], wv("cqd"), xc, start=False, stop=False)
                    nc.tensor.matmul(ps23[:M, 0:512], wv("cqd"), xp, start=False, stop=True)

                if KERNEL_V == 1:
                    a01 = sqpool.tile([128, 1024], F32, tag="a01")
                    nc.scalar.activation(a01[:M], ps01[:M, :], AF.Square)
                    a23 = sqpool.tile([128, 1024], F32, tag="a23")
                    nc.scalar.activation(a23[:M], ps23[:M, :], AF.Square)
                    return a01, a23
                if KERNEL_V == 5:
                    # Rebalance DVE->ACT: ScalarE squares the f0/f1 pair here
                    # (it is otherwise idle but for the sqrt), DVE keeps g2/g3.
                    a01 = sqpool.tile([128, 1024], F32, tag="a01")
                    nc.scalar.activation(a01[:M], ps01[:M, :], AF.Square)
                    return a01, ps23
                return ps01, ps23

            pair_state = []  # v7: [(img, r0, M)] for the pending even chunk

            def back_v6(st, chunk_idx):
                """v6 tail: ACT squares ps01 wide -> bf16, DVE native 2x adds,
                custom sqa chain on ps23 (1x, PSUM), ACT sqrt, DVE sigmoid
                poly, bf16 out. Some chunks shift g2/g3 squaring to ACT
                (type-2) to balance DVE vs ACT occupancy.

                v7: the g2/g3 chain squares independently of the f0/f1 pass
                (earlier PSUM release, shorter critical path), and sqrt/
                sigmoid/output-DMA run once per chunk PAIR at double width
                to amortize per-instruction overheads."""
                (t01, t23), (img, r0, M) = st
                # Spread type-2 chunks evenly: ci*MIX8 mod 8 < MIX8.
                type2 = (chunk_idx * KERNEL_MIX8) % 8 < KERNEL_MIX8
                sq01 = sqpool.tile([128, 1024], BF16, tag="sq01")
                nc.scalar.activation(sq01[:M], t01[:M, :], AF.Square)
                A = sqpool.tile([128, 512], BF16, tag="A")
                nc.vector.tensor_add(
                    out=A[:M], in0=sq01[:M, 0:512], in1=sq01[:M, 512:1024]
                )
                if KERNEL_V >= 7:
                    half = chunk_idx % 2
                    if half == 0:
                        sspair = sqpool.tile([128, 1024], BF16, tag="sspair")
                        pair_state.append((sspair, img, r0, M))
                    else:
                        sspair = pair_state[-1][0]
                    ss = sspair[:, 512 * half : 512 * half + 512]
                else:
                    ss_t = sqpool.tile([128, 512], BF16, tag="ss")
                    ss = ss_t[:, :]
                if type2:
                    sq23 = sqpool.tile([128, 1024], BF16, tag="sq23")
                    nc.scalar.activation(sq23[:M], t23[:M, :], AF.Square)
                    Bt = sqpool.tile([128, 512], BF16, tag="Bt")
                    nc.vector.tensor_add(
                        out=Bt[:M], in0=sq23[:M, 0:512], in1=sq23[:M, 512:1024]
                    )
                    nc.vector.tensor_add(out=ss[:M], in0=A[:M], in1=Bt[:M])
                else:
                    c3 = sqpool.tile([128, 512], BF16, tag="c3")
                    nc.vector._custom_dve(
                        sqa_op, out=c3[:M], in0=t23[:M, 0:512], in1=A[:M]
                    )
                    nc.vector._custom_dve(
                        sqa_op, out=ss[:M], in0=t23[:M, 512:1024], in1=c3[:M]
                    )
                if KERNEL_V >= 7:
                    if chunk_idx % 2 == 0:
                        return
                    sspair, img0, r00, M0 = pair_state.pop()
                    Mw = max(M0, M)
                    s = sqpool.tile([128, 1024], BF16, tag="s")
                    nc.scalar.activation(
                        s[:Mw], sspair[:Mw], AF.Sqrt, scale=1.0 / 64.0
                    )
                    rt = rpool.tile([128, 1024], BF16, tag="rt")
                    nc.vector._custom_dve(
                        sig_op,
                        out=rt[:Mw],
                        in0=s[:Mw],
                        in1=halfs2[:Mw],
                        s0=P_C1,
                        s1=P_C3,
                        imm2=P_C5,
                    )
                    nc.sync.dma_start(
                        out=y[img0, r00 : r00 + M0, :], in_=rt[:M0, 0:512]
                    )
                    nc.sync.dma_start(
                        out=y[img, r0 : r0 + M, :], in_=rt[:M, 512:1024]
                    )
                    return
                s = sqpool.tile([128, 512], BF16, tag="s")
                nc.scalar.activation(s[:M], ss[:M], AF.Sqrt, scale=1.0 / 64.0)
                rt = rpool.tile([128, 512], BF16, tag="rt")
                nc.vector._custom_dve(
                    sig_op,
                    out=rt[:M],
                    in0=s[:M],
                    in1=halfs[:M],
                    s0=P_C1,
                    s1=P_C3,
                    imm2=P_C5,
                )
                nc.sync.dma_start(out=y[img, r0 : r0 + M, :], in_=rt[:M])

            def back(st):
                """Sum of squares + sqrt (+sigmoid for v>=3), one chunk behind."""
                nonlocal last_sqrt
                (t01, t23), (img, r0, M) = st
                rt = rpool.tile([128, 512], F32, tag="rt")
                if KERNEL_V == 1:
                    u = sqpool.tile([128, 1024], F32, tag="u")
                    nc.vector.tensor_add(out=u[:M], in0=t01[:M], in1=t23[:M])
                    nc.vector.tensor_add(
                        out=rt[:M], in0=u[:M, 0:512], in1=u[:M, 512:1024]
                    )
                elif KERNEL_V == 5:
                    # t01 = ACT-squared f0/f1 pair (SBUF), t23 = raw g2/g3 PSUM.
                    s1 = sqpool.tile([128, 512], F32, tag="c1")
                    nc.vector.tensor_add(
                        out=s1[:M], in0=t01[:M, 0:512], in1=t01[:M, 512:1024]
                    )
                    c3 = sqpool.tile([128, 512], F32, tag="c3")
                    nc.vector._custom_dve(
                        sqa_op, out=c3[:M], in0=t23[:M, 0:512], in1=s1[:M]
                    )
                    nc.vector._custom_dve(
                        sqa_op, out=rt[:M], in0=t23[:M, 512:1024], in1=c3[:M]
                    )
                else:
                    # DVE custom chain: SS = f0^2 + f1^2 + g2^2 + g3^2 with one
                    # PSUM read per map and no ScalarE involvement.
                    c1 = sqpool.tile([128, 512], F32, tag="c1")
                    nc.vector._custom_dve(sq_op, out=c1[:M], in0=t01[:M, 0:512])
                    c2 = sqpool.tile([128, 512], F32, tag="c2")
                    nc.vector._custom_dve(
                        sqa_op, out=c2[:M], in0=t01[:M, 512:1024], in1=c1[:M]
                    )
                    c3 = sqpool.tile([128, 512], F32, tag="c3")
                    nc.vector._custom_dve(
                        sqa_op, out=c3[:M], in0=t23[:M, 0:512], in1=c2[:M]
                    )
                    nc.vector._custom_dve(
                        sqa_op, out=rt[:M], in0=t23[:M, 512:1024], in1=c3[:M]
                    )
                # v>=3: Sqrt(SS/64) = sqrt(SS)/8 exactly (power-of-two scale),
                # so the DVE sigmoid polynomial takes t directly.
                sqrt_scale = 1.0 / 64.0 if KERNEL_V >= 3 else 1.0
                last_sqrt = nc.scalar.activation(
                    rt[:M], rt[:M], AF.Sqrt, scale=sqrt_scale
                )
                if KERNEL_V >= 3:
                    # sigmoid(r/8) via deg-5 odd polynomial on DVE; scale 1/8
                    # folded into the ACT sqrt would disturb nothing, but the
                    # poly coefficients already absorb it via scale on sqrt.
                    nc.vector._custom_dve(
                        sig_op,
                        out=rt[:M],
                        in0=rt[:M],
                        in1=halfs[:M],
                        s0=P_C1,
                        s1=P_C3,
                        imm2=P_C5,
                    )
                    nc.sync.dma_start(out=y[img, r0 : r0 + M, :], in_=rt[:M])
                else:
                    tail.append((rt, M, img, r0))

            for rep in range(repeat):
                tail.clear()
                pending = None
                ci = 0
                for img in range(BPC):
                    for r0, M, s0, K, align in CHUNKS:
                        sq = front(img, r0, M, s0, K, align)
                        if pending is not None:
                            if KERNEL_V >= 6:
                                back_v6(pending, ci)
                            else:
                                back(pending)
                            ci += 1
                        pending = (sq, (img, r0, M))
                if KERNEL_V >= 6:
                    back_v6(pending, ci)
                else:
                    back(pending)

                # Batched sigmoids: forced after all sqrt work so only one ACT
                # table-set switch happens (sqrt_and_others -> sigmoid_and_others).
                prev = last_sqrt
                for rt, M, img, r0 in tail:
                    si = nc.scalar.activation(rt[:M], rt[:M], AF.Sigmoid, scale=0.125)
                    add_dep_helper(si.ins, prev.ins, False, "batch sigmoids after sqrts")
                    prev = si
                    nc.sync.dma_start(out=y[img, r0 : r0 + M, :], in_=rt[:M])

    nc.compile()
    return nc


_CACHE = {}


def _get_nc():
    global KERNEL_V, KERNEL_MIX8
    KERNEL_V = int(os.environ.get("KERNEL_V", "6"))
    KERNEL_MIX8 = int(os.environ.get("KERNEL_MIX8", "3"))
    repeat = int(os.environ.get("KERNEL_REPEAT", "1"))
    key = ("nc", repeat, KERNEL_V, KERNEL_MIX8)
    if key not in _CACHE:
        wts, offmap = _build_weights(pad128=KERNEL_V >= 7)
        _CACHE["wts", KERNEL_V >= 7] = wts
        _CACHE[key] = _build_nc(wts.shape[1], offmap, repeat=repeat)
    return _CACHE[key], _CACHE["wts", KERNEL_V >= 7]


def _get_nc_for_bench():
    """bench_ab hook: (nc, extra per-core inputs beyond x)."""
    nc, wts = _get_nc()
    if KERNEL_V >= 6:
        import ml_dtypes

        wts = wts.astype(ml_dtypes.bfloat16)
    return nc, {"wts": wts}


_last_result = None


def kernel(pred_mask: np.ndarray) -> np.ndarray:
    global _last_result
    from concourse.bass_utils import run_bass_kernel_spmd

    assert pred_mask.shape == (B, 1, H, W), pred_mask.shape
    nc, wts = _get_nc()
    if KERNEL_V >= 6:
        import ml_dtypes

        xs = np.ascontiguousarray(
            pred_mask.reshape(B, H, W).astype(ml_dtypes.bfloat16)
        )
        wts = wts.astype(ml_dtypes.bfloat16)
    else:
        xs = np.ascontiguousarray(pred_mask.reshape(B, H, W).astype(np.float32))
    in_maps = [
        {"x": xs[i * BPC : (i + 1) * BPC], "wts": wts} for i in range(NCORES)
    ]
    trace = bool(os.environ.get("KERNEL_TRACE"))
    res = run_bass_kernel_spmd(
        nc, in_maps, core_ids=list(range(NCORES)), trace=trace
    )
    _last_result = res
    out = np.stack([r["y"] for r in res.results], axis=0)
    return out.reshape(B, 1, H, W).astype(np.float32)

